# revision 2
# baseline (speedup 1.0000x reference)
"""Trainium2 Bass kernel: batched American-put binomial tree (n=256).

Math (matches the reference):
    v0_j = relu(k - s_term_j),  s_term_j = S0*exp(sig*sqrt(dt)*(2j - n))
    step t (t = 0..n-1, scale_t = c^t, c = exp(sig*sqrt(dt))):
        cont_j = w0*v_j + w1*v_{j+1}
        pay_j  = k - c^t * s_base_j,  s_base_j = S0*exp(sig*sqrt(dt)*(2j-(n-1)))
        v'_j   = max(cont_j, pay_j)
    answer = v[0] after n steps.

Kernel mapping (per core, 1024 strikes = 128 partitions x 8 groups):
  - state tile V[128, 8, P] (batch on partitions, tree on free dim)
  - per step, two DVE scalar_tensor_tensor ops over all 8 groups:
        U = (V[:,:,1:w+1] * (w1/w0)) + V[:,:,0:w]        # cont / w0
        V = (U * w0) max PAY[t&1][:,:,m:m+w]             # m = t>>1
    The payoff obeys pay_{t+2, j} = pay_{t, j+1} (since c^2*s_base_j =
    s_base_{j+1}), so payoffs for ALL steps are two static tiles (even/odd
    step parity) read at a sliding column offset. Zero per-step payoff cost.
  - width w_t = min(n - t, W0): beyond column W0, v_t,j == 0 for every t
    (terminal values vanish and payoff is negative there for every strike),
    so those columns are never computed and column W0 stays exactly 0.
"""

import os
import sys

for _p in ("/opt/trn_rl_repo", "/root/.axon_site/_ro/trn_rl_repo"):
    if os.path.isdir(_p) and _p not in sys.path:
        sys.path.insert(0, _p)

import numpy as np

N = 256
S0 = 100.0
SIG = 0.2
R = 0.05
DT = 1.0 / N
SQRT_DT = float(np.sqrt(DT))
U_ = float(np.exp(SIG * SQRT_DT))
D_ = float(np.exp(-SIG * SQRT_DT))
W0C = float((np.exp(-R * DT) * U_ - 1.0) / (U_ - D_))
W1C = float((1.0 - np.exp(-R * DT) * D_) / (U_ - D_))
RRATIO = W1C / W0C
C_ = U_

NCORES = 8
B = 8192
PB = B // NCORES          # rows per core
NPART = 128
NG = PB // NPART          # groups per core

_J_TERM = np.arange(N + 1, dtype=np.float64)
_S_TERM = S0 * np.exp(SIG * SQRT_DT * (2.0 * _J_TERM - N))          # (257,)
_J_IN = np.arange(N, dtype=np.float64)
_S_BASE = S0 * np.exp(SIG * SQRT_DT * (2.0 * _J_IN - (N - 1)))      # (256,)

_cache: dict = {}


def _widths(w0cap: int):
    return [min(N - t, w0cap) for t in range(N)]


def _build(w0cap: int):
    """Build + compile the Bass program for tree-width cap w0cap."""
    import concourse.bacc as bacc
    import concourse.mybir as mybir
    import concourse.tile as tile

    f32 = mybir.dt.float32
    P = w0cap + 1
    ws = _widths(w0cap)
    PW = max((t >> 1) + w for t, w in enumerate(ws))

    nc = bacc.Bacc("TRN2", target_bir_lowering=False, debug=False,
                   num_devices=NCORES)
    v0d = nc.dram_tensor("v0", [NPART, NG, P], f32, kind="ExternalInput")
    p0d = nc.dram_tensor("pay0", [NPART, NG, PW], f32, kind="ExternalInput")
    p1d = nc.dram_tensor("pay1", [NPART, NG, PW], f32, kind="ExternalInput")
    outd = nc.dram_tensor("out", [NPART, NG, 1], f32, kind="ExternalOutput")

    with tile.TileContext(nc) as tc:
        with tc.tile_pool(name="state", bufs=1) as pool:
            V = pool.tile([NPART, NG, P], f32, name="V")
            Ut = pool.tile([NPART, NG, P], f32, name="Ut")
            P0 = pool.tile([NPART, NG, PW], f32, name="P0")
            P1 = pool.tile([NPART, NG, PW], f32, name="P1")

            nc.sync.dma_start(V[:], v0d[:])
            nc.sync.dma_start(P0[:], p0d[:])
            nc.sync.dma_start(P1[:], p1d[:])

            mult = mybir.AluOpType.mult
            add = mybir.AluOpType.add
            amax = mybir.AluOpType.max
            for t in range(N):
                w = ws[t]
                m = t >> 1
                pay = P0 if (t & 1) == 0 else P1
                nc.vector.scalar_tensor_tensor(
                    Ut[:, :, 0:w], V[:, :, 1:w + 1], RRATIO, V[:, :, 0:w],
                    mult, add)
                nc.vector.scalar_tensor_tensor(
                    V[:, :, 0:w], Ut[:, :, 0:w], W0C, pay[:, :, m:m + w],
                    mult, amax)

            nc.sync.dma_start(outd[:], V[:, :, 0:1])

    nc.compile()
    return nc, P, PW


def _prep_inputs(k_flat: np.ndarray, w0cap: int, P: int, PW: int):
    """Per-core input dicts. k_flat: (B,) float32."""
    in_maps = []
    s_term = _S_TERM[:P]        # (P,)
    s_base = _S_BASE[:PW]       # (PW,)
    for c in range(NCORES):
        kc = k_flat[c * PB:(c + 1) * PB].astype(np.float64)      # (PB,)
        kc = kc.reshape(NG, NPART)                               # [g, p]
        kpg = np.ascontiguousarray(kc.T)                         # [p, g]
        v0 = np.maximum(kpg[:, :, None] - s_term[None, None, :], 0.0)
        pay0 = kpg[:, :, None] - s_base[None, None, :]
        pay1 = kpg[:, :, None] - (C_ * s_base)[None, None, :]
        in_maps.append({
            "v0": v0.astype(np.float32),
            "pay0": pay0.astype(np.float32),
            "pay1": pay1.astype(np.float32),
        })
    return in_maps


def _run(k: np.ndarray, trace: bool = False):
    from concourse.bass_utils import run_bass_kernel_spmd

    k_flat = np.asarray(k, dtype=np.float32).reshape(B)
    kmax = float(k_flat.max())
    # strict zero bound: for j >= w0cap, s_term_j >= kmax so v0_j = 0 and
    # every payoff is <= 0 (s_base_j > s_term_j, scales >= 1).
    w0cap = int(np.ceil(N / 2 + np.log(max(kmax, 1e-6) / S0)
                        / (2.0 * SIG * SQRT_DT))) + 2
    w0cap = max(1, min(N, w0cap))

    key = w0cap
    if key not in _cache:
        _cache[key] = _build(w0cap)
    nc, P, PW = _cache[key]

    in_maps = _prep_inputs(k_flat, w0cap, P, PW)
    res = run_bass_kernel_spmd(nc, in_maps, core_ids=list(range(NCORES)),
                               trace=trace)
    parts = []
    for c in range(NCORES):
        o = res.results[c]["out"][:, :, 0]          # [p, g]
        parts.append(np.ascontiguousarray(o.T).reshape(PB))
    out = np.concatenate(parts).astype(np.float32).reshape(B, 1)
    return out, res


def kernel(k: np.ndarray) -> np.ndarray:
    out, _ = _run(k, trace=False)
    return out


# revision 12
# speedup vs baseline: 1.3949x; 1.3949x over previous
"""Trainium2 Bass kernel: batched American-put binomial tree (n=256).

Math (matches the reference):
    v0_j = relu(k - s_term_j),  s_term_j = S0*exp(sig*sqrt(dt)*(2j - n))
    step t (t = 0..n-1, scale_t = c^t, c = exp(sig*sqrt(dt))):
        cont_j = w0*v_j + w1*v_{j+1}
        pay_j  = k - c^t * s_base_j,  s_base_j = S0*exp(sig*sqrt(dt)*(2j-(n-1)))
        v'_j   = max(cont_j, pay_j)
    answer = v[0] after n steps.

Kernel mapping (per core, 1024 strikes = 128 partitions x 8 groups):
  - state tile V[128, 8, P] (batch on partitions, tree on free dim)
  - per step, two DVE scalar_tensor_tensor ops over all 8 groups:
        U = (V[:,:,1:w+1] * (w1/w0)) + V[:,:,0:w]        # cont / w0
        V = (U * w0) max PAY[t&1][:,:,m:m+w]             # m = t>>1
    The payoff obeys pay_{t+2, j} = pay_{t, j+1} (since c^2*s_base_j =
    s_base_{j+1}), so payoffs for ALL steps are two static tiles (even/odd
    step parity) read at a sliding column offset. Zero per-step payoff cost.
  - width w_t = min(n - t, W0): beyond column W0, v_t,j == 0 for every t
    (terminal values vanish and payoff is negative there for every strike),
    so those columns are never computed and column W0 stays exactly 0.
"""

import os
import sys

for _p in ("/opt/trn_rl_repo", "/root/.axon_site/_ro/trn_rl_repo"):
    if os.path.isdir(_p) and _p not in sys.path:
        sys.path.insert(0, _p)

import numpy as np

N = 256
S0 = 100.0
SIG = 0.2
R = 0.05
DT = 1.0 / N
SQRT_DT = float(np.sqrt(DT))
U_ = float(np.exp(SIG * SQRT_DT))
D_ = float(np.exp(-SIG * SQRT_DT))
W0C = float((np.exp(-R * DT) * U_ - 1.0) / (U_ - D_))
W1C = float((1.0 - np.exp(-R * DT) * D_) / (U_ - D_))
RRATIO = W1C / W0C
C_ = U_

NCORES = 8
B = 8192
PB = B // NCORES          # rows per core
NPART = 128
NG = PB // NPART          # groups per core

_J_TERM = np.arange(N + 1, dtype=np.float64)
_S_TERM = S0 * np.exp(SIG * SQRT_DT * (2.0 * _J_TERM - N))          # (257,)
_J_IN = np.arange(N, dtype=np.float64)
_S_BASE = S0 * np.exp(SIG * SQRT_DT * (2.0 * _J_IN - (N - 1)))      # (256,)

_cache: dict = {}


def _widths(w0cap: int):
    return [min(N - t, w0cap) for t in range(N)]


def _trim_lo(w0cap: int, kmin: float, safety: int = 5):
    """Left-trim schedule: lo[t] = first column computed at step t.

    For j < lo[t] exercise is provably optimal for every strike in the
    batch, so v_{t+1,j} = pay_{t,j} and the kernel writes those columns
    directly from the payoff tiles (1-column strips) instead of running the
    recurrence. Rigor: the put is homogeneous (v(k,S) = k*V(S/k)) and the
    exercise region in S/k is universal, with the smallest strike the most
    restrictive row; an exact f64 recursion for kmin gives its exercise
    prefix, minus `safety` columns for f32 boundary fuzz.
    """
    ws = _widths(w0cap)
    v = np.maximum(kmin - _S_TERM, 0.0)
    lo = [0] * N
    cur = 1 << 30
    for t in range(N):
        w = ws[t]
        pay = kmin - (C_ ** t) * _S_BASE
        cont = W0C * v[:-1] + W1C * v[1:]
        exw = (pay >= cont)[:w]
        pref = int(np.argmin(exw)) if not exw.all() else w
        lo_t = max(0, min(cur, pref - safety, w - 1))
        lo[t] = lo_t
        cur = lo_t
        v = np.concatenate([np.maximum(cont, pay), v[-1:]])
    return lo


def _build(w0cap: int, pool_groups: int = 0, steps: int = N,
           lo: list | None = None):
    """Build + compile the Bass program for tree-width cap w0cap.

    pool_groups of the NG batch groups run their recurrence on the GpSimd
    (Pool) engine; the rest on the DVE. The two streams share no tiles, so
    they execute fully in parallel. When `lo` is given, columns below lo[t]
    are not computed; the freshly-exposed column strip [lo[t+1], lo[t]) is
    copied from the step-t payoff on the (otherwise idle) ACT engine.
    """
    import concourse.bacc as bacc
    import concourse.mybir as mybir
    import concourse.tile as tile

    f32 = mybir.dt.float32
    P = w0cap + 1
    ws = _widths(w0cap)
    PW = max((t >> 1) + w for t, w in enumerate(ws))
    if lo is None:
        lo = [0] * N
    gd = NG - pool_groups          # DVE groups
    gp = pool_groups

    nc = bacc.Bacc("TRN2", target_bir_lowering=False, debug=False,
                   num_devices=NCORES)
    v0d = nc.dram_tensor("v0", [NPART, NG, P], f32, kind="ExternalInput")
    p0d = nc.dram_tensor("pay0", [NPART, NG, PW], f32, kind="ExternalInput")
    p1d = nc.dram_tensor("pay1", [NPART, NG, PW], f32, kind="ExternalInput")
    outd = nc.dram_tensor("out", [NPART, NG, 1], f32, kind="ExternalOutput")

    mult = mybir.AluOpType.mult
    add = mybir.AluOpType.add
    amax = mybir.AluOpType.max

    with tile.TileContext(nc) as tc:
        with tc.tile_pool(name="state", bufs=1) as pool:
            V = pool.tile([NPART, gd, P], f32, name="V")
            Ut = pool.tile([NPART, gd, P], f32, name="Ut")
            P0 = pool.tile([NPART, gd, PW], f32, name="P0")
            P1 = pool.tile([NPART, gd, PW], f32, name="P1")
            nc.sync.dma_start(V[:], v0d[:, 0:gd, :])
            nc.sync.dma_start(P0[:], p0d[:, 0:gd, :])
            nc.sync.dma_start(P1[:], p1d[:, 0:gd, :])
            if gp:
                Vp = pool.tile([NPART, gp, P], f32, name="Vp")
                Up = pool.tile([NPART, gp, P], f32, name="Up")
                Q0 = pool.tile([NPART, gp, PW], f32, name="Q0")
                Q1 = pool.tile([NPART, gp, PW], f32, name="Q1")
                nc.sync.dma_start(Vp[:], v0d[:, gd:NG, :])
                nc.sync.dma_start(Q0[:], p0d[:, gd:NG, :])
                nc.sync.dma_start(Q1[:], p1d[:, gd:NG, :])

            copyf = mybir.ActivationFunctionType.Copy
            for t in range(steps):
                w = ws[t]
                m = t >> 1
                lt = lo[t]
                pay = P0 if (t & 1) == 0 else P1
                nc.vector.scalar_tensor_tensor(
                    Ut[:, :, lt:w], V[:, :, lt + 1:w + 1], RRATIO,
                    V[:, :, lt:w], mult, add)
                nc.vector.scalar_tensor_tensor(
                    V[:, :, lt:w], Ut[:, :, lt:w], W0C,
                    pay[:, :, m + lt:m + w], mult, amax)
                if gp:
                    payp = Q0 if (t & 1) == 0 else Q1
                    nc.gpsimd.scalar_tensor_tensor(
                        Up[:, :, lt:w], Vp[:, :, lt + 1:w + 1], RRATIO,
                        Vp[:, :, lt:w], mult, add)
                    nc.gpsimd.scalar_tensor_tensor(
                        Vp[:, :, lt:w], Up[:, :, lt:w], W0C,
                        payp[:, :, m + lt:m + w], mult, amax)
                # expose columns the next step reads below lo[t]: they hold
                # v_{t+1} = pay_t there (exercise region)
                if t + 1 < steps and lo[t + 1] < lt:
                    s0, s1 = lo[t + 1], lt
                    nc.scalar.activation(
                        V[:, :, s0:s1], pay[:, :, m + s0:m + s1], copyf)
                    if gp:
                        nc.scalar.activation(
                            Vp[:, :, s0:s1], payp[:, :, m + s0:m + s1],
                            copyf)

            nc.sync.dma_start(outd[:, 0:gd, :], V[:, :, 0:1])
            if gp:
                nc.sync.dma_start(outd[:, gd:NG, :], Vp[:, :, 0:1])

    nc.compile()
    return nc, P, PW


def _build_pe(w0cap: int, steps: int = N, lo: list | None = None):
    """PE+DVE variant: the TensorEngine computes cont = w0*v_j + w1*v_{j+1}
    into PSUM via two accumulating matmuls with diagonal weights; the DVE
    only does max(cont, pay) + eviction to SBUF. Two independent 4-group
    halves (A/B) pipeline the PE<->DVE dependency chain.
    """
    import concourse.bacc as bacc
    import concourse.mybir as mybir
    import concourse.tile as tile

    f32 = mybir.dt.float32
    P = w0cap + 1
    ws = _widths(w0cap)
    PW = max((t >> 1) + w for t, w in enumerate(ws))
    if lo is None:
        lo = [0] * N
    GH = NG // 2      # groups per half

    nc = bacc.Bacc("TRN2", target_bir_lowering=False, debug=False,
                   num_devices=NCORES)
    v0d = nc.dram_tensor("v0", [NPART, NG, P], f32, kind="ExternalInput")
    p0d = nc.dram_tensor("pay0", [NPART, NG, PW], f32, kind="ExternalInput")
    p1d = nc.dram_tensor("pay1", [NPART, NG, PW], f32, kind="ExternalInput")
    wgtd = nc.dram_tensor("wdiag", [2, NPART, NPART], f32,
                          kind="ExternalInput")
    outd = nc.dram_tensor("out", [NPART, NG, 1], f32, kind="ExternalOutput")

    amax = mybir.AluOpType.max
    copyf = mybir.ActivationFunctionType.Copy

    with tile.TileContext(nc) as tc:
        with (tc.tile_pool(name="state", bufs=1) as pool,
              tc.tile_pool(name="psum", bufs=2, space="PSUM") as pspool):
            W0D = pool.tile([NPART, NPART], f32, name="W0D")
            W1D = pool.tile([NPART, NPART], f32, name="W1D")
            nc.sync.dma_start(W0D[:], wgtd[0])
            nc.sync.dma_start(W1D[:], wgtd[1])

            halves = []
            for h in range(2):
                g0 = h * GH
                V = pool.tile([NPART, GH, P], f32, name=f"V{h}")
                PA = pool.tile([NPART, GH, PW], f32, name=f"PA{h}")
                PB = pool.tile([NPART, GH, PW], f32, name=f"PB{h}")
                nc.sync.dma_start(V[:], v0d[:, g0:g0 + GH, :])
                nc.sync.dma_start(PA[:], p0d[:, g0:g0 + GH, :])
                nc.sync.dma_start(PB[:], p1d[:, g0:g0 + GH, :])
                halves.append((V, PA, PB))

            for t in range(steps):
                w = ws[t]
                m = t >> 1
                lt = lo[t]
                wl = w - lt
                for h, (V, PA, PB) in enumerate(halves):
                    pay = PA if (t & 1) == 0 else PB
                    ps = pspool.tile([NPART, 512], f32, tag=f"ps{h}",
                                     name=f"ps{h}_{t}")
                    nc.tensor.matmul(ps[:, 0:GH * wl], W0D[:],
                                     V[:, :, lt:w], start=True, stop=False)
                    nc.tensor.matmul(ps[:, 0:GH * wl], W1D[:],
                                     V[:, :, lt + 1:w + 1], start=False,
                                     stop=True)
                    ps3 = ps[:, 0:GH * wl].rearrange("p (g x) -> p g x", g=GH)
                    nc.vector.tensor_tensor(
                        V[:, :, lt:w], ps3, pay[:, :, m + lt:m + w], amax)
                    if t + 1 < steps and lo[t + 1] < lt:
                        s0, s1 = lo[t + 1], lt
                        nc.scalar.activation(
                            V[:, :, s0:s1], pay[:, :, m + s0:m + s1], copyf)

            for h, (V, _, _) in enumerate(halves):
                g0 = h * GH
                nc.sync.dma_start(outd[:, g0:g0 + GH, :], V[:, :, 0:1])

    nc.compile()
    return nc, P, PW


def _prep_inputs(k_flat: np.ndarray, w0cap: int, P: int, PW: int):
    """Per-core input dicts. k_flat: (B,) float32."""
    in_maps = []
    s_term = _S_TERM[:P]        # (P,)
    s_base = _S_BASE[:PW]       # (PW,)
    for c in range(NCORES):
        kc = k_flat[c * PB:(c + 1) * PB].astype(np.float64)      # (PB,)
        kc = kc.reshape(NG, NPART)                               # [g, p]
        kpg = np.ascontiguousarray(kc.T)                         # [p, g]
        v0 = np.maximum(kpg[:, :, None] - s_term[None, None, :], 0.0)
        pay0 = kpg[:, :, None] - s_base[None, None, :]
        pay1 = kpg[:, :, None] - (C_ * s_base)[None, None, :]
        in_maps.append({
            "v0": v0.astype(np.float32),
            "pay0": pay0.astype(np.float32),
            "pay1": pay1.astype(np.float32),
        })
    return in_maps


def _run(k: np.ndarray, trace: bool = False):
    from concourse.bass_utils import run_bass_kernel_spmd

    k_flat = np.asarray(k, dtype=np.float32).reshape(B)
    kmax = float(k_flat.max())
    # strict zero bound: for j >= w0cap, s_term_j >= kmax so v0_j = 0 and
    # every payoff is <= 0 (s_base_j > s_term_j, scales >= 1).
    w0cap = int(np.ceil(N / 2 + np.log(max(kmax, 1e-6) / S0)
                        / (2.0 * SIG * SQRT_DT))) + 2
    w0cap = max(1, min(N, w0cap))

    pool_groups = int(os.environ.get("BT_POOL_GROUPS", "0"))
    trim = os.environ.get("BT_TRIM", "1") == "1"
    lo = _trim_lo(w0cap, float(k_flat.min())) if trim else None
    key = (w0cap, pool_groups, tuple(lo) if lo else None)
    if key not in _cache:
        _cache[key] = _build(w0cap, pool_groups, lo=lo)
    nc, P, PW = _cache[key]

    in_maps = _prep_inputs(k_flat, w0cap, P, PW)
    res = run_bass_kernel_spmd(nc, in_maps, core_ids=list(range(NCORES)),
                               trace=trace)
    parts = []
    for c in range(NCORES):
        o = res.results[c]["out"][:, :, 0]          # [p, g]
        parts.append(np.ascontiguousarray(o.T).reshape(PB))
    out = np.concatenate(parts).astype(np.float32).reshape(B, 1)
    return out, res


def kernel(k: np.ndarray) -> np.ndarray:
    out, _ = _run(k, trace=False)
    return out


# revision 17
# speedup vs baseline: 6408.9696x; 4594.6479x over previous
"""Trainium2 Bass kernel: batched American-put binomial tree (n=256).

Math (matches the reference):
    v0_j = relu(k - s_term_j),  s_term_j = S0*exp(sig*sqrt(dt)*(2j - n))
    step t (t = 0..n-1, scale_t = c^t, c = exp(sig*sqrt(dt))):
        cont_j = w0*v_j + w1*v_{j+1}
        pay_j  = k - c^t * s_base_j,  s_base_j = S0*exp(sig*sqrt(dt)*(2j-(n-1)))
        v'_j   = max(cont_j, pay_j)
    answer = v[0] after n steps.

Kernel mapping (per core, 1024 strikes = 128 partitions x 8 groups):
  - state tile V[128, 8, P] (batch on partitions, tree on free dim)
  - per step, two DVE scalar_tensor_tensor ops over all 8 groups:
        U = (V[:,:,1:w+1] * (w1/w0)) + V[:,:,0:w]        # cont / w0
        V = (U * w0) max PAY[t&1][:,:,m:m+w]             # m = t>>1
    The payoff obeys pay_{t+2, j} = pay_{t, j+1} (since c^2*s_base_j =
    s_base_{j+1}), so payoffs for ALL steps are two static tiles (even/odd
    step parity) read at a sliding column offset. Zero per-step payoff cost.
  - width w_t = min(n - t, W0): beyond column W0, v_t,j == 0 for every t
    (terminal values vanish and payoff is negative there for every strike),
    so those columns are never computed and column W0 stays exactly 0.
"""

import os
import sys

for _p in ("/opt/trn_rl_repo", "/root/.axon_site/_ro/trn_rl_repo"):
    if os.path.isdir(_p) and _p not in sys.path:
        sys.path.insert(0, _p)

import numpy as np

N = 256
S0 = 100.0
SIG = 0.2
R = 0.05
DT = 1.0 / N
SQRT_DT = float(np.sqrt(DT))
U_ = float(np.exp(SIG * SQRT_DT))
D_ = float(np.exp(-SIG * SQRT_DT))
W0C = float((np.exp(-R * DT) * U_ - 1.0) / (U_ - D_))
W1C = float((1.0 - np.exp(-R * DT) * D_) / (U_ - D_))
RRATIO = W1C / W0C
C_ = U_

NCORES = 8
B = 8192
PB = B // NCORES          # rows per core
NPART = 128
NG = PB // NPART          # groups per core

_J_TERM = np.arange(N + 1, dtype=np.float64)
_S_TERM = S0 * np.exp(SIG * SQRT_DT * (2.0 * _J_TERM - N))          # (257,)
_J_IN = np.arange(N, dtype=np.float64)
_S_BASE = S0 * np.exp(SIG * SQRT_DT * (2.0 * _J_IN - (N - 1)))      # (256,)

_cache: dict = {}


def _widths(w0cap: int):
    return [min(N - t, w0cap) for t in range(N)]


def _trim_lo(w0cap: int, kmin: float, safety: int = 5):
    """Left-trim schedule: lo[t] = first column computed at step t.

    For j < lo[t] exercise is provably optimal for every strike in the
    batch, so v_{t+1,j} = pay_{t,j} and the kernel writes those columns
    directly from the payoff tiles (1-column strips) instead of running the
    recurrence. Rigor: the put is homogeneous (v(k,S) = k*V(S/k)) and the
    exercise region in S/k is universal, with the smallest strike the most
    restrictive row; an exact f64 recursion for kmin gives its exercise
    prefix, minus `safety` columns for f32 boundary fuzz.
    """
    ws = _widths(w0cap)
    v = np.maximum(kmin - _S_TERM, 0.0)
    lo = [0] * N
    cur = 1 << 30
    for t in range(N):
        w = ws[t]
        pay = kmin - (C_ ** t) * _S_BASE
        cont = W0C * v[:-1] + W1C * v[1:]
        exw = (pay >= cont)[:w]
        pref = int(np.argmin(exw)) if not exw.all() else w
        lo_t = max(0, min(cur, pref - safety, w - 1))
        lo[t] = lo_t
        cur = lo_t
        v = np.concatenate([np.maximum(cont, pay), v[-1:]])
    return lo


def _build(w0cap: int, pool_groups: int = 0, steps: int = N,
           lo: list | None = None, reps: int = 1):
    """Build + compile the Bass program for tree-width cap w0cap.

    pool_groups of the NG batch groups run their recurrence on the GpSimd
    (Pool) engine; the rest on the DVE. The two streams share no tiles, so
    they execute fully in parallel. When `lo` is given, columns below lo[t]
    are not computed; the freshly-exposed column strip [lo[t+1], lo[t]) is
    copied from the step-t payoff on the (otherwise idle) ACT engine.
    """
    import concourse.bacc as bacc
    import concourse.mybir as mybir
    import concourse.tile as tile

    f32 = mybir.dt.float32
    P = w0cap + 1
    ws = _widths(w0cap)
    PW = max((t >> 1) + w for t, w in enumerate(ws))
    if lo is None:
        lo = [0] * N
    gd = NG - pool_groups          # DVE groups
    gp = pool_groups

    nc = bacc.Bacc("TRN2", target_bir_lowering=False, debug=False,
                   num_devices=NCORES)
    v0d = nc.dram_tensor("v0", [NPART, NG, P], f32, kind="ExternalInput")
    p0d = nc.dram_tensor("pay0", [NPART, NG, PW], f32, kind="ExternalInput")
    p1d = nc.dram_tensor("pay1", [NPART, NG, PW], f32, kind="ExternalInput")
    outd = nc.dram_tensor("out", [NPART, NG, 1], f32, kind="ExternalOutput")

    mult = mybir.AluOpType.mult
    add = mybir.AluOpType.add
    amax = mybir.AluOpType.max

    with tile.TileContext(nc) as tc:
        with tc.tile_pool(name="state", bufs=1) as pool:
            V = pool.tile([NPART, gd, P], f32, name="V")
            Ut = pool.tile([NPART, gd, P], f32, name="Ut")
            P0 = pool.tile([NPART, gd, PW], f32, name="P0")
            P1 = pool.tile([NPART, gd, PW], f32, name="P1")
            nc.sync.dma_start(P0[:], p0d[:, 0:gd, :])
            nc.sync.dma_start(P1[:], p1d[:, 0:gd, :])
            if gp:
                Vp = pool.tile([NPART, gp, P], f32, name="Vp")
                Up = pool.tile([NPART, gp, P], f32, name="Up")
                Q0 = pool.tile([NPART, gp, PW], f32, name="Q0")
                Q1 = pool.tile([NPART, gp, PW], f32, name="Q1")
                nc.sync.dma_start(Vp[:], v0d[:, gd:NG, :])
                nc.sync.dma_start(Q0[:], p0d[:, gd:NG, :])
                nc.sync.dma_start(Q1[:], p1d[:, gd:NG, :])

            copyf = mybir.ActivationFunctionType.Copy
            for _rep in range(reps):
              nc.sync.dma_start(V[:], v0d[:, 0:gd, :])
              if gp:
                nc.sync.dma_start(Vp[:], v0d[:, gd:NG, :])
              for t in range(steps):
                w = ws[t]
                m = t >> 1
                lt = lo[t]
                pay = P0 if (t & 1) == 0 else P1
                nc.vector.scalar_tensor_tensor(
                    Ut[:, :, lt:w], V[:, :, lt + 1:w + 1], RRATIO,
                    V[:, :, lt:w], mult, add)
                nc.vector.scalar_tensor_tensor(
                    V[:, :, lt:w], Ut[:, :, lt:w], W0C,
                    pay[:, :, m + lt:m + w], mult, amax)
                if gp:
                    payp = Q0 if (t & 1) == 0 else Q1
                    nc.gpsimd.scalar_tensor_tensor(
                        Up[:, :, lt:w], Vp[:, :, lt + 1:w + 1], RRATIO,
                        Vp[:, :, lt:w], mult, add)
                    nc.gpsimd.scalar_tensor_tensor(
                        Vp[:, :, lt:w], Up[:, :, lt:w], W0C,
                        payp[:, :, m + lt:m + w], mult, amax)
                # expose columns the next step reads below lo[t]: they hold
                # v_{t+1} = pay_t there (exercise region)
                if t + 1 < steps and lo[t + 1] < lt:
                    s0, s1 = lo[t + 1], lt
                    nc.scalar.activation(
                        V[:, :, s0:s1], pay[:, :, m + s0:m + s1], copyf)
                    if gp:
                        nc.scalar.activation(
                            Vp[:, :, s0:s1], payp[:, :, m + s0:m + s1],
                            copyf)

            nc.sync.dma_start(outd[:, 0:gd, :], V[:, :, 0:1])
            if gp:
                nc.sync.dma_start(outd[:, gd:NG, :], Vp[:, :, 0:1])

    nc.compile()
    return nc, P, PW


def _build_pe(w0cap: int, steps: int = N, lo: list | None = None):
    """PE+DVE variant: the TensorEngine computes cont = w0*v_j + w1*v_{j+1}
    into PSUM via two accumulating matmuls with diagonal weights; the DVE
    only does max(cont, pay) + eviction to SBUF. Two independent 4-group
    halves (A/B) pipeline the PE<->DVE dependency chain.
    """
    import concourse.bacc as bacc
    import concourse.mybir as mybir
    import concourse.tile as tile

    f32 = mybir.dt.float32
    P = w0cap + 1
    ws = _widths(w0cap)
    PW = max((t >> 1) + w for t, w in enumerate(ws))
    if lo is None:
        lo = [0] * N
    GH = NG // 2      # groups per half

    nc = bacc.Bacc("TRN2", target_bir_lowering=False, debug=False,
                   num_devices=NCORES)
    v0d = nc.dram_tensor("v0", [NPART, NG, P], f32, kind="ExternalInput")
    p0d = nc.dram_tensor("pay0", [NPART, NG, PW], f32, kind="ExternalInput")
    p1d = nc.dram_tensor("pay1", [NPART, NG, PW], f32, kind="ExternalInput")
    wgtd = nc.dram_tensor("wdiag", [2, NPART, NPART], f32,
                          kind="ExternalInput")
    outd = nc.dram_tensor("out", [NPART, NG, 1], f32, kind="ExternalOutput")

    amax = mybir.AluOpType.max
    copyf = mybir.ActivationFunctionType.Copy

    with tile.TileContext(nc) as tc:
        with (tc.tile_pool(name="state", bufs=1) as pool,
              tc.tile_pool(name="psum", bufs=2, space="PSUM") as pspool):
            W0D = pool.tile([NPART, NPART], f32, name="W0D")
            W1D = pool.tile([NPART, NPART], f32, name="W1D")
            nc.sync.dma_start(W0D[:], wgtd[0])
            nc.sync.dma_start(W1D[:], wgtd[1])

            halves = []
            for h in range(2):
                g0 = h * GH
                V = pool.tile([NPART, GH, P], f32, name=f"V{h}")
                PA = pool.tile([NPART, GH, PW], f32, name=f"PA{h}")
                PB = pool.tile([NPART, GH, PW], f32, name=f"PB{h}")
                nc.sync.dma_start(V[:], v0d[:, g0:g0 + GH, :])
                nc.sync.dma_start(PA[:], p0d[:, g0:g0 + GH, :])
                nc.sync.dma_start(PB[:], p1d[:, g0:g0 + GH, :])
                halves.append((V, PA, PB))

            for t in range(steps):
                w = ws[t]
                m = t >> 1
                lt = lo[t]
                wl = w - lt
                for h, (V, PA, PB) in enumerate(halves):
                    pay = PA if (t & 1) == 0 else PB
                    ps = pspool.tile([NPART, 512], f32, tag=f"ps{h}",
                                     name=f"ps{h}_{t}")
                    nc.tensor.matmul(ps[:, 0:GH * wl], W0D[:],
                                     V[:, :, lt:w], start=True, stop=False)
                    nc.tensor.matmul(ps[:, 0:GH * wl], W1D[:],
                                     V[:, :, lt + 1:w + 1], start=False,
                                     stop=True)
                    ps3 = ps[:, 0:GH * wl].rearrange("p (g x) -> p g x", g=GH)
                    nc.vector.tensor_tensor(
                        V[:, :, lt:w], ps3, pay[:, :, m + lt:m + w], amax)
                    if t + 1 < steps and lo[t + 1] < lt:
                        s0, s1 = lo[t + 1], lt
                        nc.scalar.activation(
                            V[:, :, s0:s1], pay[:, :, m + s0:m + s1], copyf)

            for h, (V, _, _) in enumerate(halves):
                g0 = h * GH
                nc.sync.dma_start(outd[:, g0:g0 + GH, :], V[:, :, 0:1])

    nc.compile()
    return nc, P, PW


def _prep_inputs(k_flat: np.ndarray, w0cap: int, P: int, PW: int):
    """Per-core input dicts. k_flat: (B,) float32."""
    in_maps = []
    s_term = _S_TERM[:P]        # (P,)
    s_base = _S_BASE[:PW]       # (PW,)
    for c in range(NCORES):
        kc = k_flat[c * PB:(c + 1) * PB].astype(np.float64)      # (PB,)
        kc = kc.reshape(NG, NPART)                               # [g, p]
        kpg = np.ascontiguousarray(kc.T)                         # [p, g]
        v0 = np.maximum(kpg[:, :, None] - s_term[None, None, :], 0.0)
        pay0 = kpg[:, :, None] - s_base[None, None, :]
        pay1 = kpg[:, :, None] - (C_ * s_base)[None, None, :]
        in_maps.append({
            "v0": v0.astype(np.float32),
            "pay0": pay0.astype(np.float32),
            "pay1": pay1.astype(np.float32),
        })
    return in_maps


def _run(k: np.ndarray, trace: bool = False):
    from concourse.bass_utils import run_bass_kernel_spmd

    k_flat = np.asarray(k, dtype=np.float32).reshape(B)
    kmax = float(k_flat.max())
    # strict zero bound: for j >= w0cap, s_term_j >= kmax so v0_j = 0 and
    # every payoff is <= 0 (s_base_j > s_term_j, scales >= 1).
    w0cap = int(np.ceil(N / 2 + np.log(max(kmax, 1e-6) / S0)
                        / (2.0 * SIG * SQRT_DT))) + 2
    w0cap = max(1, min(N, w0cap))

    pool_groups = int(os.environ.get("BT_POOL_GROUPS", "0"))
    use_pe = os.environ.get("BT_PE", "0") == "1"
    trim = os.environ.get("BT_TRIM", "1") == "1"
    lo = _trim_lo(w0cap, float(k_flat.min())) if trim else None
    key = (w0cap, pool_groups, use_pe, tuple(lo) if lo else None)
    if key not in _cache:
        if use_pe:
            _cache[key] = _build_pe(w0cap, lo=lo)
        else:
            _cache[key] = _build(w0cap, pool_groups, lo=lo)
    nc, P, PW = _cache[key]

    in_maps = _prep_inputs(k_flat, w0cap, P, PW)
    if use_pe:
        wd = np.zeros((2, NPART, NPART), np.float32)
        np.fill_diagonal(wd[0], np.float32(W0C))
        np.fill_diagonal(wd[1], np.float32(W1C))
        for m in in_maps:
            m["wdiag"] = wd
    res = run_bass_kernel_spmd(nc, in_maps, core_ids=list(range(NCORES)),
                               trace=trace)
    parts = []
    for c in range(NCORES):
        o = res.results[c]["out"][:, :, 0]          # [p, g]
        parts.append(np.ascontiguousarray(o.T).reshape(PB))
    out = np.concatenate(parts).astype(np.float32).reshape(B, 1)
    return out, res


def kernel(k: np.ndarray) -> np.ndarray:
    out, _ = _run(k, trace=False)
    return out
